# revision 7
# baseline (speedup 1.0000x reference)
"""DaConA-style dense MLP recommender kernel for 8 Trainium2 NeuronCores.

Algorithm (matches the fp32 jax reference):
  u_c = user_inter[rows];  i_c = item_inter[cols]          gathers, [B, 960]
  tu  = u_c @ Wt.T + bt;   ti  = i_c @ Wt.T + bt           transform, 960x960
  factor = [u_s, i_s, tu * ti]                              [B, 1024]
  3x (tanh o Linear)  ->  pred = factor @ Wr.T + br + 3.5   [B, 1]

Distribution: pure data parallelism.  Each of the 8 cores receives the
full tables + weights and 1/8 of the batch (16384 rows).

On-chip dataflow (everything feature-major, batch on the free axis):
  - indirect-DMA gather of 128 table rows per call -> [128 batch, 992] tiles
    (tables are host-concatenated [inter(960) | indep(32)] so one gather
    fetches both the 960-dim interaction row and the 32-dim indep row)
  - PE-transpose (identity matmul) into [feature, batch] k-tiles
  - transform matmuls in fp32r (full PE rate at free dim 512), PSUM accum
  - DVE: +bt bias, Hadamard tu*ti -> factor tiles
  - MLP matmuls in fp32r, tanh on the scalar engine with per-partition bias
  - final 1-row matmul with Wr, +br +3.5, DMA out
"""

import sys

sys.path.insert(0, "/opt/trn_rl_repo")

import numpy as np

import concourse.bass as bass
import concourse.mybir as mybir
import concourse.tile as tile
from concourse.bass import IndirectOffsetOnAxis
from concourse.bass_utils import run_bass_kernel_spmd
from concourse.masks import make_identity

N_CORES = 8
BATCH = 131072
BC = BATCH // N_CORES            # 16384 rows per core
NB = 512                         # batch tile (PSUM bank width in fp32)
N_USERS, N_ITEMS = 100000, 50000
DIM_C = 960                      # interaction feature dim
DIM_S = 32                       # indep feature dim
DIM_CAT = DIM_C + DIM_S          # 992 = gathered row width
D1, D2, D3 = 512, 256, 128       # MLP widths
GLOBAL_AVG = 3.5

F32 = mybir.dt.float32
F32R = mybir.dt.float32r
I32 = mybir.dt.int32

# feature k-tiles of the 992-wide gathered rows: 7 x 128 + 96
G_TILES = [(ft, 128 if ft < 7 else 96) for ft in range(8)]
# contraction k-tiles of the 960-wide transform: 7 x 128 + 64
K_TILES = [(kt, 128 if kt < 7 else 64) for kt in range(8)]
# output m-tiles of the 960-wide transform: 7 x 128 + 64
M_TILES = K_TILES


def _fix_drains(nc):
    """This walrus build only encodes one sync-wait per instruction for
    several opcode variants (Drain, self-loading Matmult, ...): "Too many
    sync wait commands".  Hoist all-but-one wait of any multi-wait
    instruction onto single-wait EventSemaphore nops placed just before it
    on the same engine — semantically identical (waits are processed
    in-order by the engine's sequencer before dispatch)."""
    for bb in nc.main_func.blocks:
        insts = list(bb.instructions)
        out_list = []
        changed = False
        for ins in insts:
            si = ins.sync_info
            if si is not None and len(si.on_wait) > 1:
                for k, w in enumerate(si.on_wait[:-1]):
                    es = mybir.InstEventSemaphore(
                        name=f"{ins.name}_dw{k}", ins=[], outs=[]
                    )
                    es.engine = ins.engine
                    es.sync_info = mybir.SyncInfo(on_wait=[w], on_update=[])
                    out_list.append(es)
                ins.sync_info = mybir.SyncInfo(
                    on_wait=[si.on_wait[-1]], on_update=list(si.on_update)
                )
                changed = True
            out_list.append(ins)
        if changed:
            bb.instructions = out_list


def build_nc(n_users=N_USERS, n_items=N_ITEMS, bc=BC, fix_drains=True):
    """Trace the per-core SPMD program. bc = batch rows on this core."""
    nbt = bc // NB                       # number of batch tiles
    n_gather = bc // 128                 # indirect-gather calls per table
    mm = bass.mybir.AluOpType

    nc = bass.Bass(target_bir_lowering=False, debug=False, trn_type="TRN2")

    rows_d = nc.dram_tensor("rows_t", [128, n_gather], I32, kind="ExternalInput")
    cols_d = nc.dram_tensor("cols_t", [128, n_gather], I32, kind="ExternalInput")
    tab_u = nc.dram_tensor("tab_u", [n_users, DIM_CAT], F32, kind="ExternalInput")
    tab_i = nc.dram_tensor("tab_i", [n_items, DIM_CAT], F32, kind="ExternalInput")
    wtT_d = nc.dram_tensor("wtT", [DIM_C, DIM_C], F32, kind="ExternalInput")
    w1T_d = nc.dram_tensor("w1T", [2 * D1, D1], F32, kind="ExternalInput")
    w2T_d = nc.dram_tensor("w2T", [D1, D2], F32, kind="ExternalInput")
    w3T_d = nc.dram_tensor("w3T", [D2, D3], F32, kind="ExternalInput")
    wrT_d = nc.dram_tensor("wrT", [D3, 1], F32, kind="ExternalInput")
    btT_d = nc.dram_tensor("btT", [128, 8], F32, kind="ExternalInput")
    b1T_d = nc.dram_tensor("b1T", [128, 4], F32, kind="ExternalInput")
    b2T_d = nc.dram_tensor("b2T", [128, 2], F32, kind="ExternalInput")
    b3T_d = nc.dram_tensor("b3T", [128, 1], F32, kind="ExternalInput")
    br_d = nc.dram_tensor("br", [1, 1], F32, kind="ExternalInput")
    out_d = nc.dram_tensor("out", [bc], F32, kind="ExternalOutput")

    with tile.TileContext(nc) as tc:
        with (
            tc.tile_pool(name="wpool", bufs=1) as wp,
            tc.tile_pool(name="raw", bufs=6) as rawp,
            tc.tile_pool(name="feat", bufs=1) as fp,
            tc.tile_pool(name="act", bufs=1) as hp,
            tc.tile_pool(name="outp", bufs=2) as op,
            tc.tile_pool(name="pstp", bufs=3, space="PSUM") as pstp,
            tc.tile_pool(name="psmm", bufs=2, space="PSUM") as psmm,
        ):
            # ---- persistent weights / indices / identity ----
            ident = wp.tile([128, 128], F32, tag="ident")
            make_identity(nc, ident[:])

            rows_sb = wp.tile([128, n_gather], I32, tag="rows")
            cols_sb = wp.tile([128, n_gather], I32, tag="cols")
            nc.sync.dma_start(rows_sb[:], rows_d[:])
            nc.sync.dma_start(cols_sb[:], cols_d[:])

            wt_sb = []
            for kt, kw in K_TILES:
                t = wp.tile([128, DIM_C], F32R, tag=f"wt{kt}")
                nc.sync.dma_start(t[:kw, :], wtT_d[kt * 128 : kt * 128 + kw, :].bitcast(F32R))
                wt_sb.append(t)
            w1_sb = []
            for kt in range(8):
                t = wp.tile([128, D1], F32R, tag=f"w1{kt}")
                nc.sync.dma_start(t[:], w1T_d[kt * 128 : (kt + 1) * 128, :].bitcast(F32R))
                w1_sb.append(t)
            w2_sb = []
            for kt in range(4):
                t = wp.tile([128, D2], F32R, tag=f"w2{kt}")
                nc.sync.dma_start(t[:], w2T_d[kt * 128 : (kt + 1) * 128, :].bitcast(F32R))
                w2_sb.append(t)
            w3_sb = []
            for kt in range(2):
                t = wp.tile([128, D3], F32R, tag=f"w3{kt}")
                nc.sync.dma_start(t[:], w3T_d[kt * 128 : (kt + 1) * 128, :].bitcast(F32R))
                w3_sb.append(t)
            wr_sb = wp.tile([128, 1], F32R, tag="wr")
            nc.sync.dma_start(wr_sb[:], wrT_d[:].bitcast(F32R))
            btT = wp.tile([128, 8], F32, tag="btT")
            nc.sync.dma_start(btT[:], btT_d[:])
            b1T = wp.tile([128, 4], F32, tag="b1T")
            nc.sync.dma_start(b1T[:], b1T_d[:])
            b2T = wp.tile([128, 2], F32, tag="b2T")
            nc.sync.dma_start(b2T[:], b2T_d[:])
            b3T = wp.tile([128, 1], F32, tag="b3T")
            nc.sync.dma_start(b3T[:], b3T_d[:])
            br_sb = wp.tile([1, 1], F32, tag="br")
            nc.sync.dma_start(br_sb[:], br_d[:])

            # ---- batch loop ----
            for t in range(nbt):
                # gather: 4 blocks of 128 rows per table
                u_raw, i_raw = [], []
                for blk in range(4):
                    g = 4 * t + blk
                    ur = rawp.tile([128, DIM_CAT], F32, tag="uraw")
                    nc.gpsimd.indirect_dma_start(
                        out=ur[:],
                        out_offset=None,
                        in_=tab_u[:],
                        in_offset=IndirectOffsetOnAxis(ap=rows_sb[:, g : g + 1], axis=0),
                    )
                    u_raw.append(ur)
                    ir = rawp.tile([128, DIM_CAT], F32, tag="iraw")
                    nc.gpsimd.indirect_dma_start(
                        out=ir[:],
                        out_offset=None,
                        in_=tab_i[:],
                        in_offset=IndirectOffsetOnAxis(ap=cols_sb[:, g : g + 1], axis=0),
                    )
                    i_raw.append(ir)

                # transpose gathered tiles to feature-major k-tiles
                u_t, i_t = [], []
                for raws, dst, tg in ((u_raw, u_t, "ut"), (i_raw, i_t, "it")):
                    for ft, pw in G_TILES:
                        ftile = fp.tile([128, NB], F32R, tag=f"{tg}{ft}")
                        for blk in range(4):
                            tp = pstp.tile([128, 128], F32, tag="tp")
                            nc.tensor.transpose(
                                out=tp[:pw, :],
                                in_=raws[blk][:, ft * 128 : ft * 128 + pw],
                                identity=ident[:],
                            )
                            nc.vector.tensor_copy(
                                out=ftile[:pw, blk * 128 : (blk + 1) * 128],
                                in_=tp[:pw, :],
                            )
                        dst.append(ftile)

                # transform matmuls (fp32r) + bias + Hadamard -> factor tiles
                factor = []
                for ft in range(8):
                    factor.append(fp.tile([128, NB], F32R, tag=f"fac{ft}", name=f"fac{ft}"))
                for mt, mw in M_TILES:
                    tu_ps = psmm.tile([128, NB], F32, tag="mmA")
                    ti_ps = psmm.tile([128, NB], F32, tag="mmB")
                    for kt, kw in K_TILES:
                        la = wt_sb[kt][:kw, mt * 128 : mt * 128 + mw]
                        nc.tensor.matmul(
                            tu_ps[:mw, :], lhsT=la, rhs=u_t[kt][:kw, :],
                            start=(kt == 0), stop=(kt == 7),
                        )
                    for kt, kw in K_TILES:
                        la = wt_sb[kt][:kw, mt * 128 : mt * 128 + mw]
                        nc.tensor.matmul(
                            ti_ps[:mw, :], lhsT=la, rhs=i_t[kt][:kw, :],
                            start=(kt == 0), stop=(kt == 7),
                        )
                    tu_sb = op.tile([128, NB], F32, tag="tub")
                    nc.vector.tensor_scalar(
                        out=tu_sb[:mw, :], in0=tu_ps[:mw, :],
                        scalar1=btT[:mw, mt : mt + 1], scalar2=None, op0=mm.add,
                    )
                    nc.vector.tensor_scalar(
                        out=ti_ps[:mw, :], in0=ti_ps[:mw, :],
                        scalar1=btT[:mw, mt : mt + 1], scalar2=None, op0=mm.add,
                    )
                    nc.vector.tensor_tensor(
                        out=factor[mt][:mw, :], in0=tu_sb[:mw, :], in1=ti_ps[:mw, :],
                        op=mm.mult,
                    )
                # indep features ride in partitions 64:96 of gathered tile 7
                nc.vector.tensor_copy(out=factor[7][64:96, :], in_=u_t[7][64:96, :])
                nc.vector.tensor_copy(out=factor[7][96:128, :], in_=i_t[7][64:96, :])

                # MLP layer 1: 1024 -> 512, tanh
                h1 = []
                for mt in range(4):
                    ps = psmm.tile([128, NB], F32, tag="mmA")
                    for kt in range(8):
                        nc.tensor.matmul(
                            ps[:],
                            lhsT=w1_sb[kt][:, mt * 128 : (mt + 1) * 128],
                            rhs=factor[kt][:],
                            start=(kt == 0), stop=(kt == 7),
                        )
                    h = hp.tile([128, NB], F32R, tag=f"h1{mt}")
                    nc.scalar.activation(
                        h[:], ps[:], mybir.ActivationFunctionType.Tanh,
                        bias=b1T[:, mt : mt + 1],
                    )
                    h1.append(h)

                # MLP layer 2: 512 -> 256, tanh
                h2 = []
                for mt in range(2):
                    ps = psmm.tile([128, NB], F32, tag="mmB")
                    for kt in range(4):
                        nc.tensor.matmul(
                            ps[:],
                            lhsT=w2_sb[kt][:, mt * 128 : (mt + 1) * 128],
                            rhs=h1[kt][:],
                            start=(kt == 0), stop=(kt == 3),
                        )
                    h = hp.tile([128, NB], F32R, tag=f"h2{mt}")
                    nc.scalar.activation(
                        h[:], ps[:], mybir.ActivationFunctionType.Tanh,
                        bias=b2T[:, mt : mt + 1],
                    )
                    h2.append(h)

                # MLP layer 3: 256 -> 128, tanh
                ps = psmm.tile([128, NB], F32, tag="mmA")
                for kt in range(2):
                    nc.tensor.matmul(
                        ps[:],
                        lhsT=w3_sb[kt][:],
                        rhs=h2[kt][:],
                        start=(kt == 0), stop=(kt == 1),
                    )
                h3 = hp.tile([128, NB], F32R, tag="h3")
                nc.scalar.activation(
                    h3[:], ps[:], mybir.ActivationFunctionType.Tanh, bias=b3T[:, 0:1]
                )

                # regression head: 128 -> 1, + br + 3.5
                pp = psmm.tile([128, NB], F32, tag="mmB")
                nc.tensor.matmul(
                    pp[:1, :], lhsT=wr_sb[:, :1], rhs=h3[:],
                    start=True, stop=True,
                )
                pred = op.tile([1, NB], F32, tag="pred")
                nc.vector.tensor_scalar(
                    out=pred[:], in0=pp[:1, :], scalar1=br_sb[:1, 0:1],
                    scalar2=GLOBAL_AVG, op0=mm.add, op1=mm.add,
                )
                nc.sync.dma_start(out=out_d[t * NB : (t + 1) * NB], in_=pred[:1, :])

    if fix_drains:
        _fix_drains(nc)
    return nc


def _host_prep(rows, cols, user_inter, item_inter, user_indep_x, item_indep_x,
               Wt, bt, W1, b1, W2, b2, W3, b3, Wr, br, n_cores=N_CORES):
    """Build the shared (weights/tables) and per-core (indices) input maps."""
    f32 = np.float32
    tab_u = np.ascontiguousarray(
        np.concatenate([user_inter, user_indep_x], axis=1), dtype=f32
    )
    tab_i = np.ascontiguousarray(
        np.concatenate([item_inter, item_indep_x], axis=1), dtype=f32
    )
    wtT = np.ascontiguousarray(np.asarray(Wt, f32).T)
    # factor layout is [inter(960), u_s(32), i_s(32)] -> permute W1 columns
    W1 = np.asarray(W1, f32)
    w1p = np.concatenate([W1[:, 64:], W1[:, :32], W1[:, 32:64]], axis=1)
    w1T = np.ascontiguousarray(w1p.T)
    w2T = np.ascontiguousarray(np.asarray(W2, f32).T)
    w3T = np.ascontiguousarray(np.asarray(W3, f32).T)
    wrT = np.ascontiguousarray(np.asarray(Wr, f32).T)

    def padT(v, ntiles):
        v = np.asarray(v, f32)
        out = np.zeros((128, ntiles), f32)
        out.flat[: 0] = 0  # keep shape
        for t in range(ntiles):
            seg = v[t * 128 : (t + 1) * 128]
            out[: len(seg), t] = seg
        return out

    btT = padT(bt, 8)
    b1T = padT(b1, 4)
    b2T = padT(b2, 2)
    b3T = padT(b3, 1)
    brv = np.asarray(br, f32).reshape(1, 1)

    shared = dict(tab_u=tab_u, tab_i=tab_i, wtT=wtT, w1T=w1T, w2T=w2T, w3T=w3T,
                  wrT=wrT, btT=btT, b1T=b1T, b2T=b2T, b3T=b3T, br=brv)

    bc = len(rows) // n_cores
    in_maps = []
    for c in range(n_cores):
        rc = np.asarray(rows[c * bc : (c + 1) * bc], np.int32)
        cc = np.asarray(cols[c * bc : (c + 1) * bc], np.int32)
        m = dict(shared)
        m["rows_t"] = np.ascontiguousarray(rc.reshape(-1, 128).T)
        m["cols_t"] = np.ascontiguousarray(cc.reshape(-1, 128).T)
        in_maps.append(m)
    return in_maps


def kernel(rows, cols, user_inter, item_inter, user_indep_x, item_indep_x,
           Wt, bt, W1, b1, W2, b2, W3, b3, Wr, br):
    in_maps = _host_prep(rows, cols, user_inter, item_inter, user_indep_x,
                         item_indep_x, Wt, bt, W1, b1, W2, b2, W3, b3, Wr, br)
    nc = build_nc()
    res = run_bass_kernel_spmd(nc, in_maps, list(range(N_CORES)))
    out = np.concatenate([res.results[c]["out"] for c in range(N_CORES)])
    return out.reshape(BATCH, 1).astype(np.float32)


# revision 10
# speedup vs baseline: 1.2333x; 1.2333x over previous
"""DaConA-style dense MLP recommender kernel for 8 Trainium2 NeuronCores.

Algorithm (matches the fp32 jax reference):
  u_c = user_inter[rows];  i_c = item_inter[cols]          gathers, [B, 960]
  tu  = u_c @ Wt.T + bt;   ti  = i_c @ Wt.T + bt           transform, 960x960
  factor = [u_s, i_s, tu * ti]                              [B, 1024]
  3x (tanh o Linear)  ->  pred = factor @ Wr.T + br + 3.5   [B, 1]

Distribution: pure data parallelism.  Each of the 8 cores receives the
full tables + weights and 1/8 of the batch (16384 rows).

On-chip dataflow (everything feature-major, batch on the free axis):
  - indirect-DMA gather of 128 table rows per call -> [128 batch, 992] tiles
    (tables are host-concatenated [inter(960) | indep(32)] so one gather
    fetches both the 960-dim interaction row and the 32-dim indep row)
  - PE-transpose (identity matmul) into [feature, batch] k-tiles
  - transform matmuls in fp32r (full PE rate at free dim 512), PSUM accum
  - DVE: +bt bias, Hadamard tu*ti -> factor tiles
  - MLP matmuls in fp32r, tanh on the scalar engine with per-partition bias
  - final 1-row matmul with Wr, +br +3.5, DMA out
"""

import sys

sys.path.insert(0, "/opt/trn_rl_repo")

import numpy as np

import concourse.bass as bass
import concourse.mybir as mybir
import concourse.tile as tile
from concourse.bass import IndirectOffsetOnAxis
from concourse.bass_utils import run_bass_kernel_spmd
from concourse.masks import make_identity

N_CORES = 8
BATCH = 131072
BC = BATCH // N_CORES            # 16384 rows per core
NB = 512                         # batch tile (PSUM bank width in fp32)
N_USERS, N_ITEMS = 100000, 50000
DIM_C = 960                      # interaction feature dim
DIM_S = 32                       # indep feature dim
DIM_CAT = DIM_C + DIM_S          # 992 = gathered row width
D1, D2, D3 = 512, 256, 128       # MLP widths
GLOBAL_AVG = 3.5

F32 = mybir.dt.float32
F32R = mybir.dt.float32r
BF16 = mybir.dt.bfloat16
I32 = mybir.dt.int32
USE_BF16 = True                  # bf16 matmul pipeline (2x PE rate, 2x less gather DMA)

# feature k-tiles of the 992-wide gathered rows: 7 x 128 + 96
G_TILES = [(ft, 128 if ft < 7 else 96) for ft in range(8)]
# contraction k-tiles of the 960-wide transform: 7 x 128 + 64
K_TILES = [(kt, 128 if kt < 7 else 64) for kt in range(8)]
# output m-tiles of the 960-wide transform: 7 x 128 + 64
M_TILES = K_TILES


def _fix_drains(nc):
    """This walrus build only encodes one sync-wait per instruction for
    several opcode variants (Drain, self-loading Matmult, ...): "Too many
    sync wait commands".  Hoist all-but-one wait of any multi-wait
    instruction onto single-wait EventSemaphore nops placed just before it
    on the same engine — semantically identical (waits are processed
    in-order by the engine's sequencer before dispatch)."""
    for bb in nc.main_func.blocks:
        insts = list(bb.instructions)
        out_list = []
        changed = False
        for ins in insts:
            si = ins.sync_info
            if si is not None and len(si.on_wait) > 1:
                for k, w in enumerate(si.on_wait[:-1]):
                    es = mybir.InstEventSemaphore(
                        name=f"{ins.name}_dw{k}", ins=[], outs=[]
                    )
                    es.engine = ins.engine
                    es.sync_info = mybir.SyncInfo(on_wait=[w], on_update=[])
                    out_list.append(es)
                ins.sync_info = mybir.SyncInfo(
                    on_wait=[si.on_wait[-1]], on_update=list(si.on_update)
                )
                changed = True
            out_list.append(ins)
        if changed:
            bb.instructions = out_list


def build_nc(n_users=N_USERS, n_items=N_ITEMS, bc=BC, fix_drains=True, use_bf16=USE_BF16):
    """Trace the per-core SPMD program. bc = batch rows on this core."""
    nbt = bc // NB                       # number of batch tiles
    n_gather = bc // 128                 # indirect-gather calls per table
    mm = bass.mybir.AluOpType

    TD = BF16 if use_bf16 else F32           # table / gathered dtype
    WD = BF16 if use_bf16 else F32R          # weight / matmul-feed dtype
    nc = bass.Bass(target_bir_lowering=False, debug=False, trn_type="TRN2")

    rows_d = nc.dram_tensor("rows_t", [128, n_gather], I32, kind="ExternalInput")
    cols_d = nc.dram_tensor("cols_t", [128, n_gather], I32, kind="ExternalInput")
    tab_u = nc.dram_tensor("tab_u", [n_users, DIM_CAT], TD, kind="ExternalInput")
    tab_i = nc.dram_tensor("tab_i", [n_items, DIM_CAT], TD, kind="ExternalInput")
    wtT_d = nc.dram_tensor("wtT", [DIM_C, DIM_C], WD, kind="ExternalInput")
    w1T_d = nc.dram_tensor("w1T", [2 * D1, D1], WD, kind="ExternalInput")
    w2T_d = nc.dram_tensor("w2T", [D1, D2], WD, kind="ExternalInput")
    w3T_d = nc.dram_tensor("w3T", [D2, D3], WD, kind="ExternalInput")
    wrT_d = nc.dram_tensor("wrT", [D3, 1], WD, kind="ExternalInput")
    btT_d = nc.dram_tensor("btT", [128, 8], F32, kind="ExternalInput")
    b1T_d = nc.dram_tensor("b1T", [128, 4], F32, kind="ExternalInput")
    b2T_d = nc.dram_tensor("b2T", [128, 2], F32, kind="ExternalInput")
    b3T_d = nc.dram_tensor("b3T", [128, 1], F32, kind="ExternalInput")
    br_d = nc.dram_tensor("br", [1, 1], F32, kind="ExternalInput")
    out_d = nc.dram_tensor("out", [bc], F32, kind="ExternalOutput")

    with tile.TileContext(nc) as tc:
        with (
            tc.tile_pool(name="wpool", bufs=1) as wp,
            tc.tile_pool(name="raw", bufs=6) as rawp,
            tc.tile_pool(name="feat", bufs=1) as fp,
            tc.tile_pool(name="act", bufs=1) as hp,
            tc.tile_pool(name="outp", bufs=2) as op,
            tc.tile_pool(name="pstp", bufs=3, space="PSUM") as pstp,
            tc.tile_pool(name="psmm", bufs=2, space="PSUM") as psmm,
        ):
            # ---- persistent weights / indices / identity ----
            ident = wp.tile([128, 128], TD, tag="ident")
            make_identity(nc, ident[:])

            rows_sb = wp.tile([128, n_gather], I32, tag="rows")
            cols_sb = wp.tile([128, n_gather], I32, tag="cols")
            nc.sync.dma_start(rows_sb[:], rows_d[:])
            nc.sync.dma_start(cols_sb[:], cols_d[:])

            wt_sb = []
            for kt, kw in K_TILES:
                t = wp.tile([128, DIM_C], WD, tag=f"wt{kt}")
                nc.sync.dma_start(t[:kw, :], wtT_d[kt * 128 : kt * 128 + kw, :])
                wt_sb.append(t)
            w1_sb = []
            for kt in range(8):
                t = wp.tile([128, D1], WD, tag=f"w1{kt}")
                nc.sync.dma_start(t[:], w1T_d[kt * 128 : (kt + 1) * 128, :])
                w1_sb.append(t)
            w2_sb = []
            for kt in range(4):
                t = wp.tile([128, D2], WD, tag=f"w2{kt}")
                nc.sync.dma_start(t[:], w2T_d[kt * 128 : (kt + 1) * 128, :])
                w2_sb.append(t)
            w3_sb = []
            for kt in range(2):
                t = wp.tile([128, D3], WD, tag=f"w3{kt}")
                nc.sync.dma_start(t[:], w3T_d[kt * 128 : (kt + 1) * 128, :])
                w3_sb.append(t)
            wr_sb = wp.tile([128, 1], WD, tag="wr")
            nc.sync.dma_start(wr_sb[:], wrT_d[:])
            btT = wp.tile([128, 8], F32, tag="btT")
            nc.sync.dma_start(btT[:], btT_d[:])
            b1T = wp.tile([128, 4], F32, tag="b1T")
            nc.sync.dma_start(b1T[:], b1T_d[:])
            b2T = wp.tile([128, 2], F32, tag="b2T")
            nc.sync.dma_start(b2T[:], b2T_d[:])
            b3T = wp.tile([128, 1], F32, tag="b3T")
            nc.sync.dma_start(b3T[:], b3T_d[:])
            br_sb = wp.tile([1, 1], F32, tag="br")
            nc.sync.dma_start(br_sb[:], br_d[:])

            # ---- batch loop ----
            for t in range(nbt):
                # gather: 4 blocks of 128 rows per table
                u_raw, i_raw = [], []
                for blk in range(4):
                    g = 4 * t + blk
                    ur = rawp.tile([128, DIM_CAT], TD, tag="uraw")
                    nc.gpsimd.indirect_dma_start(
                        out=ur[:],
                        out_offset=None,
                        in_=tab_u[:],
                        in_offset=IndirectOffsetOnAxis(ap=rows_sb[:, g : g + 1], axis=0),
                    )
                    u_raw.append(ur)
                    ir = rawp.tile([128, DIM_CAT], TD, tag="iraw")
                    nc.gpsimd.indirect_dma_start(
                        out=ir[:],
                        out_offset=None,
                        in_=tab_i[:],
                        in_offset=IndirectOffsetOnAxis(ap=cols_sb[:, g : g + 1], axis=0),
                    )
                    i_raw.append(ir)

                # transpose gathered tiles to feature-major k-tiles
                u_t, i_t = [], []
                for raws, dst, tg in ((u_raw, u_t, "ut"), (i_raw, i_t, "it")):
                    for ft, pw in G_TILES:
                        ftile = fp.tile([128, NB], WD, tag=f"{tg}{ft}")
                        for blk in range(4):
                            tp = pstp.tile([128, 128], TD, tag="tp")
                            nc.tensor.transpose(
                                out=tp[:pw, :],
                                in_=raws[blk][:, ft * 128 : ft * 128 + pw],
                                identity=ident[:],
                            )
                            nc.vector.tensor_copy(
                                out=ftile[:pw, blk * 128 : (blk + 1) * 128],
                                in_=tp[:pw, :],
                            )
                        dst.append(ftile)

                # transform matmuls (fp32r) + bias + Hadamard -> factor tiles
                factor = []
                for ft in range(8):
                    factor.append(fp.tile([128, NB], WD, tag=f"fac{ft}", name=f"fac{ft}"))
                for mt, mw in M_TILES:
                    tu_ps = psmm.tile([128, NB], F32, tag="mmA")
                    ti_ps = psmm.tile([128, NB], F32, tag="mmB")
                    for kt, kw in K_TILES:
                        la = wt_sb[kt][:kw, mt * 128 : mt * 128 + mw]
                        nc.tensor.matmul(
                            tu_ps[:mw, :], lhsT=la, rhs=u_t[kt][:kw, :],
                            start=(kt == 0), stop=(kt == 7),
                        )
                    for kt, kw in K_TILES:
                        la = wt_sb[kt][:kw, mt * 128 : mt * 128 + mw]
                        nc.tensor.matmul(
                            ti_ps[:mw, :], lhsT=la, rhs=i_t[kt][:kw, :],
                            start=(kt == 0), stop=(kt == 7),
                        )
                    tu_sb = op.tile([128, NB], F32, tag="tub")
                    nc.vector.tensor_scalar(
                        out=tu_sb[:mw, :], in0=tu_ps[:mw, :],
                        scalar1=btT[:mw, mt : mt + 1], scalar2=None, op0=mm.add,
                    )
                    nc.vector.tensor_scalar(
                        out=ti_ps[:mw, :], in0=ti_ps[:mw, :],
                        scalar1=btT[:mw, mt : mt + 1], scalar2=None, op0=mm.add,
                    )
                    nc.vector.tensor_tensor(
                        out=factor[mt][:mw, :], in0=tu_sb[:mw, :], in1=ti_ps[:mw, :],
                        op=mm.mult,
                    )
                # indep features ride in partitions 64:96 of gathered tile 7
                nc.vector.tensor_copy(out=factor[7][64:96, :], in_=u_t[7][64:96, :])
                nc.vector.tensor_copy(out=factor[7][96:128, :], in_=i_t[7][64:96, :])

                # MLP layer 1: 1024 -> 512, tanh
                h1 = []
                for mt in range(4):
                    ps = psmm.tile([128, NB], F32, tag="mmA")
                    for kt in range(8):
                        nc.tensor.matmul(
                            ps[:],
                            lhsT=w1_sb[kt][:, mt * 128 : (mt + 1) * 128],
                            rhs=factor[kt][:],
                            start=(kt == 0), stop=(kt == 7),
                        )
                    h = hp.tile([128, NB], WD, tag=f"h1{mt}")
                    nc.scalar.activation(
                        h[:], ps[:], mybir.ActivationFunctionType.Tanh,
                        bias=b1T[:, mt : mt + 1],
                    )
                    h1.append(h)

                # MLP layer 2: 512 -> 256, tanh
                h2 = []
                for mt in range(2):
                    ps = psmm.tile([128, NB], F32, tag="mmB")
                    for kt in range(4):
                        nc.tensor.matmul(
                            ps[:],
                            lhsT=w2_sb[kt][:, mt * 128 : (mt + 1) * 128],
                            rhs=h1[kt][:],
                            start=(kt == 0), stop=(kt == 3),
                        )
                    h = hp.tile([128, NB], WD, tag=f"h2{mt}")
                    nc.scalar.activation(
                        h[:], ps[:], mybir.ActivationFunctionType.Tanh,
                        bias=b2T[:, mt : mt + 1],
                    )
                    h2.append(h)

                # MLP layer 3: 256 -> 128, tanh
                ps = psmm.tile([128, NB], F32, tag="mmA")
                for kt in range(2):
                    nc.tensor.matmul(
                        ps[:],
                        lhsT=w3_sb[kt][:],
                        rhs=h2[kt][:],
                        start=(kt == 0), stop=(kt == 1),
                    )
                h3 = hp.tile([128, NB], WD, tag="h3")
                nc.scalar.activation(
                    h3[:], ps[:], mybir.ActivationFunctionType.Tanh, bias=b3T[:, 0:1]
                )

                # regression head: 128 -> 1, + br + 3.5
                pp = psmm.tile([128, NB], F32, tag="mmB")
                nc.tensor.matmul(
                    pp[:1, :], lhsT=wr_sb[:, :1], rhs=h3[:],
                    start=True, stop=True,
                )
                pred = op.tile([1, NB], F32, tag="pred")
                nc.vector.tensor_scalar(
                    out=pred[:], in0=pp[:1, :], scalar1=br_sb[:1, 0:1],
                    scalar2=GLOBAL_AVG, op0=mm.add, op1=mm.add,
                )
                nc.sync.dma_start(out=out_d[t * NB : (t + 1) * NB], in_=pred[:1, :])

    if fix_drains:
        _fix_drains(nc)
    return nc


def _host_prep(rows, cols, user_inter, item_inter, user_indep_x, item_indep_x,
               Wt, bt, W1, b1, W2, b2, W3, b3, Wr, br, n_cores=N_CORES,
               use_bf16=USE_BF16):
    """Build the shared (weights/tables) and per-core (indices) input maps."""
    f32 = np.float32
    if use_bf16:
        import ml_dtypes
        md = ml_dtypes.bfloat16
    else:
        md = f32
    tab_u = np.ascontiguousarray(
        np.concatenate([user_inter, user_indep_x], axis=1), dtype=md
    )
    tab_i = np.ascontiguousarray(
        np.concatenate([item_inter, item_indep_x], axis=1), dtype=md
    )
    wtT = np.ascontiguousarray(np.asarray(Wt, f32).T.astype(md))
    # factor layout is [inter(960), u_s(32), i_s(32)] -> permute W1 columns
    W1 = np.asarray(W1, f32)
    w1p = np.concatenate([W1[:, 64:], W1[:, :32], W1[:, 32:64]], axis=1)
    w1T = np.ascontiguousarray(w1p.T.astype(md))
    w2T = np.ascontiguousarray(np.asarray(W2, f32).T.astype(md))
    w3T = np.ascontiguousarray(np.asarray(W3, f32).T.astype(md))
    wrT = np.ascontiguousarray(np.asarray(Wr, f32).T.astype(md))

    def padT(v, ntiles):
        v = np.asarray(v, f32)
        out = np.zeros((128, ntiles), f32)
        out.flat[: 0] = 0  # keep shape
        for t in range(ntiles):
            seg = v[t * 128 : (t + 1) * 128]
            out[: len(seg), t] = seg
        return out

    btT = padT(bt, 8)
    b1T = padT(b1, 4)
    b2T = padT(b2, 2)
    b3T = padT(b3, 1)
    brv = np.asarray(br, f32).reshape(1, 1)

    shared = dict(tab_u=tab_u, tab_i=tab_i, wtT=wtT, w1T=w1T, w2T=w2T, w3T=w3T,
                  wrT=wrT, btT=btT, b1T=b1T, b2T=b2T, b3T=b3T, br=brv)

    bc = len(rows) // n_cores
    in_maps = []
    for c in range(n_cores):
        rc = np.asarray(rows[c * bc : (c + 1) * bc], np.int32)
        cc = np.asarray(cols[c * bc : (c + 1) * bc], np.int32)
        m = dict(shared)
        m["rows_t"] = np.ascontiguousarray(rc.reshape(-1, 128).T)
        m["cols_t"] = np.ascontiguousarray(cc.reshape(-1, 128).T)
        in_maps.append(m)
    return in_maps


def kernel(rows, cols, user_inter, item_inter, user_indep_x, item_indep_x,
           Wt, bt, W1, b1, W2, b2, W3, b3, Wr, br):
    in_maps = _host_prep(rows, cols, user_inter, item_inter, user_indep_x,
                         item_indep_x, Wt, bt, W1, b1, W2, b2, W3, b3, Wr, br)
    nc = build_nc()
    res = run_bass_kernel_spmd(nc, in_maps, list(range(N_CORES)))
    out = np.concatenate([res.results[c]["out"] for c in range(N_CORES)])
    return out.reshape(BATCH, 1).astype(np.float32)


# revision 13
# speedup vs baseline: 1.2402x; 1.0056x over previous
"""DaConA-style dense MLP recommender kernel for 8 Trainium2 NeuronCores.

Algorithm (matches the fp32 jax reference):
  u_c = user_inter[rows];  i_c = item_inter[cols]          gathers, [B, 960]
  tu  = u_c @ Wt.T + bt;   ti  = i_c @ Wt.T + bt           transform, 960x960
  factor = [u_s, i_s, tu * ti]                              [B, 1024]
  3x (tanh o Linear)  ->  pred = factor @ Wr.T + br + 3.5   [B, 1]

Distribution: pure data parallelism; each core gets the full tables +
weights and 1/8 of the (bucket-reordered) batch.

Dataflow: tables are host-packed to [inter(960) | indep(32) | pad] = 1024
bf16 columns.  `dma_gather(transpose=True)` fetches 128..512 rows per call
and lands them feature-major in SBUF ([128 partitions, 8 k-tiles, n batch])
— the matmul-ready layout, no on-chip transposes.  The transform runs in
bf16 at full PE rate (free dim up to 512), accumulating fp32 in PSUM; DVE
applies +bt and the Hadamard product; the 3-layer MLP runs in bf16 with
tanh (+bias) on the scalar engine; a final 1-row matmul with Wr produces
the prediction (+br +3.5).

dma_gather indices are int16, so table rows are addressed within 32768-row
chunks.  The host sorts the whole batch by (user-chunk, item-chunk) bucket,
pads each bucket to a multiple of 8*128, and deals equal 128-row groups to
every core — all 8 cores see the identical static group structure, keeping
the program SPMD.  The final [B,1] output is un-permuted on the host.
"""

import sys

sys.path.insert(0, "/opt/trn_rl_repo")

import numpy as np

import concourse.bass as bass
import concourse.mybir as mybir
import concourse.tile as tile
from concourse.bass_utils import run_bass_kernel_spmd

N_CORES = 8
BATCH = 131072
NB = 512                         # batch tile (PSUM bank width in fp32)
N_USERS, N_ITEMS = 100000, 50000
DIM_C = 960                      # interaction feature dim
DIM_S = 32                       # indep feature dim
DIM_P = 1024                     # padded gathered row width (bf16, 2048B)
D1, D2, D3 = 512, 256, 128       # MLP widths
GLOBAL_AVG = 3.5
CHUNK = 32768                    # int16 index window

F32 = mybir.dt.float32
BF16 = mybir.dt.bfloat16
I16 = mybir.dt.int16

# contraction k-tiles of the 960-wide transform: 7 x 128 + 64
K_TILES = [(kt, 128 if kt < 7 else 64) for kt in range(8)]
M_TILES = K_TILES


def _fix_drains(nc):
    """This walrus build only encodes one sync-wait per instruction for
    several opcode variants (Drain, self-loading Matmult, ...): "Too many
    sync wait commands".  Hoist all-but-one wait of any multi-wait
    instruction onto single-wait EventSemaphore nops placed just before it
    on the same engine — semantically identical (waits are processed
    in-order by the engine's sequencer before dispatch)."""
    for bb in nc.main_func.blocks:
        insts = list(bb.instructions)
        out_list = []
        changed = False
        for ins in insts:
            si = ins.sync_info
            if si is not None and len(si.on_wait) > 1:
                for k, w in enumerate(si.on_wait[:-1]):
                    es = mybir.InstEventSemaphore(
                        name=f"{ins.name}_dw{k}", ins=[], outs=[]
                    )
                    es.engine = ins.engine
                    es.sync_info = mybir.SyncInfo(on_wait=[w], on_update=[])
                    out_list.append(es)
                ins.sync_info = mybir.SyncInfo(
                    on_wait=[si.on_wait[-1]], on_update=list(si.on_update)
                )
                changed = True
            out_list.append(ins)
        if changed:
            bb.instructions = out_list


def _runs(vals):
    """[(val, start, count)] for consecutive equal entries."""
    out = []
    for j, v in enumerate(vals):
        if out and out[-1][0] == v:
            out[-1][2] += 1
        else:
            out.append([v, j, 1])
    return [tuple(r) for r in out]


def build_nc(groups, n_users=N_USERS, n_items=N_ITEMS, fix_drains=True):
    """Trace the per-core SPMD program.

    groups: per-128-row-group (user_chunk, item_chunk) ids — identical on
    every core; len(groups) % 4 == 0; bc = 128 * len(groups)."""
    assert len(groups) % 4 == 0
    nbt = len(groups) // 4
    bc = 128 * len(groups)
    mm = bass.mybir.AluOpType

    nc = bass.Bass(target_bir_lowering=False, debug=False, trn_type="TRN2")

    rows_d = nc.dram_tensor("rows16", [128, bc // 16], I16, kind="ExternalInput")
    cols_d = nc.dram_tensor("cols16", [128, bc // 16], I16, kind="ExternalInput")
    tab_u = nc.dram_tensor("tab_u", [n_users, DIM_P], BF16, kind="ExternalInput")
    tab_i = nc.dram_tensor("tab_i", [n_items, DIM_P], BF16, kind="ExternalInput")
    wtT_d = nc.dram_tensor("wtT", [DIM_C, DIM_C], BF16, kind="ExternalInput")
    w1T_d = nc.dram_tensor("w1T", [2 * D1, D1], BF16, kind="ExternalInput")
    w2T_d = nc.dram_tensor("w2T", [D1, D2], BF16, kind="ExternalInput")
    w3T_d = nc.dram_tensor("w3T", [D2, D3], BF16, kind="ExternalInput")
    wrT_d = nc.dram_tensor("wrT", [D3, 1], BF16, kind="ExternalInput")
    btT_d = nc.dram_tensor("btT", [128, 8], F32, kind="ExternalInput")
    b1T_d = nc.dram_tensor("b1T", [128, 4], F32, kind="ExternalInput")
    b2T_d = nc.dram_tensor("b2T", [128, 2], F32, kind="ExternalInput")
    b3T_d = nc.dram_tensor("b3T", [128, 1], F32, kind="ExternalInput")
    br_d = nc.dram_tensor("br", [1, 1], F32, kind="ExternalInput")
    out_d = nc.dram_tensor("out", [bc], F32, kind="ExternalOutput")

    with tile.TileContext(nc) as tc:
        with (
            tc.tile_pool(name="wpool", bufs=1) as wp,
            tc.tile_pool(name="gath", bufs=6) as gp,
            tc.tile_pool(name="feat", bufs=2) as fp,
            tc.tile_pool(name="act", bufs=2) as hp,
            tc.tile_pool(name="outp", bufs=2) as op,
            tc.tile_pool(name="psmm", bufs=3, space="PSUM") as psmm,
        ):
            # ---- persistent weights / indices ----
            rows_sb = wp.tile([128, bc // 16], I16, tag="rows")
            cols_sb = wp.tile([128, bc // 16], I16, tag="cols")
            nc.sync.dma_start(rows_sb[:], rows_d[:])
            nc.sync.dma_start(cols_sb[:], cols_d[:])

            wt_sb = []
            for kt, kw in K_TILES:
                t = wp.tile([128, DIM_C], BF16, tag=f"wt{kt}")
                nc.sync.dma_start(t[:kw, :], wtT_d[kt * 128 : kt * 128 + kw, :])
                wt_sb.append(t)
            w1_sb = []
            for kt in range(8):
                t = wp.tile([128, D1], BF16, tag=f"w1{kt}")
                nc.sync.dma_start(t[:], w1T_d[kt * 128 : (kt + 1) * 128, :])
                w1_sb.append(t)
            w2_sb = []
            for kt in range(4):
                t = wp.tile([128, D2], BF16, tag=f"w2{kt}")
                nc.sync.dma_start(t[:], w2T_d[kt * 128 : (kt + 1) * 128, :])
                w2_sb.append(t)
            w3_sb = []
            for kt in range(2):
                t = wp.tile([128, D3], BF16, tag=f"w3{kt}")
                nc.sync.dma_start(t[:], w3T_d[kt * 128 : (kt + 1) * 128, :])
                w3_sb.append(t)
            wr_sb = wp.tile([128, 1], BF16, tag="wr")
            nc.sync.dma_start(wr_sb[:], wrT_d[:])
            btT = wp.tile([128, 8], F32, tag="btT")
            nc.sync.dma_start(btT[:], btT_d[:])
            b1T = wp.tile([128, 4], F32, tag="b1T")
            nc.sync.dma_start(b1T[:], b1T_d[:])
            b2T = wp.tile([128, 2], F32, tag="b2T")
            nc.sync.dma_start(b2T[:], b2T_d[:])
            b3T = wp.tile([128, 1], F32, tag="b3T")
            nc.sync.dma_start(b3T[:], b3T_d[:])
            br_sb = wp.tile([1, 1], F32, tag="br")
            nc.sync.dma_start(br_sb[:], br_d[:])

            def gather_subs(tab_d, n_rows, idx_sb, runs, t, tag):
                """One transposed dma_gather per chunk-run of this batch
                tile; returns [(tile, off, n)] with feature-major layout
                [128, 8 k-tiles, n]."""
                subs = []
                for ck, goff, gcnt in runs:
                    n = gcnt * 128
                    off = goff * 128
                    base = ck * CHUNK
                    span = min(CHUNK, n_rows - base)
                    g = gp.tile([128, 8 * NB], BF16, tag=tag, name=f"{tag}{t}")
                    o16 = (t * NB + off) // 16
                    nc.gpsimd.dma_gather(
                        out_ap=g[:, : 8 * n].rearrange("p (c n) -> p c n", c=8),
                        in_ap=tab_d[base : base + span, :],
                        idxs_ap=idx_sb[:, o16 : o16 + n // 16],
                        num_idxs=n,
                        num_idxs_reg=n,
                        elem_size=DIM_P,
                        transpose=True,
                    )
                    subs.append((g, off, n))
                return subs

            # ---- batch loop ----
            for t in range(nbt):
                gt = groups[4 * t : 4 * t + 4]
                u_subs = gather_subs(tab_u, n_users, rows_sb,
                                     _runs([g[0] for g in gt]), t, "gu")
                i_subs = gather_subs(tab_i, n_items, cols_sb,
                                     _runs([g[1] for g in gt]), t, "gi")

                # transform matmuls (bf16) + bias + Hadamard -> factor tiles
                factor = []
                for ft in range(8):
                    factor.append(fp.tile([128, NB], BF16, tag=f"fac{ft}",
                                          name=f"fac{ft}"))
                for mt, mw in M_TILES:
                    tu_ps = psmm.tile([128, NB], F32, tag="mmA")
                    ti_ps = psmm.tile([128, NB], F32, tag="mmB")
                    for subs, ps in ((u_subs, tu_ps), (i_subs, ti_ps)):
                        for g, off, n in subs:
                            for kt, kw in K_TILES:
                                nc.tensor.matmul(
                                    ps[:mw, off : off + n],
                                    lhsT=wt_sb[kt][:kw, mt * 128 : mt * 128 + mw],
                                    rhs=g[:kw, kt * n : (kt + 1) * n],
                                    start=(kt == 0), stop=(kt == 7),
                                )
                    tu_sb = op.tile([128, NB], F32, tag="tub")
                    nc.vector.tensor_scalar(
                        out=tu_sb[:mw, :], in0=tu_ps[:mw, :],
                        scalar1=btT[:mw, mt : mt + 1], scalar2=None, op0=mm.add,
                    )
                    nc.vector.tensor_scalar(
                        out=ti_ps[:mw, :], in0=ti_ps[:mw, :],
                        scalar1=btT[:mw, mt : mt + 1], scalar2=None, op0=mm.add,
                    )
                    nc.vector.tensor_tensor(
                        out=factor[mt][:mw, :], in0=tu_sb[:mw, :], in1=ti_ps[:mw, :],
                        op=mm.mult,
                    )
                # indep features live at k-tile 7, partitions 64:96
                for g, off, n in u_subs:
                    nc.vector.tensor_copy(out=factor[7][64:96, off : off + n],
                                          in_=g[64:96, 7 * n : 7 * n + n])
                for g, off, n in i_subs:
                    nc.vector.tensor_copy(out=factor[7][96:128, off : off + n],
                                          in_=g[64:96, 7 * n : 7 * n + n])

                # MLP layer 1: 1024 -> 512, tanh
                h1 = []
                for mt in range(4):
                    ps = psmm.tile([128, NB], F32, tag="mmA")
                    for kt in range(8):
                        nc.tensor.matmul(
                            ps[:],
                            lhsT=w1_sb[kt][:, mt * 128 : (mt + 1) * 128],
                            rhs=factor[kt][:],
                            start=(kt == 0), stop=(kt == 7),
                        )
                    h = hp.tile([128, NB], BF16, tag=f"h1{mt}")
                    nc.scalar.activation(
                        h[:], ps[:], mybir.ActivationFunctionType.Tanh,
                        bias=b1T[:, mt : mt + 1],
                    )
                    h1.append(h)

                # MLP layer 2: 512 -> 256, tanh
                h2 = []
                for mt in range(2):
                    ps = psmm.tile([128, NB], F32, tag="mmB")
                    for kt in range(4):
                        nc.tensor.matmul(
                            ps[:],
                            lhsT=w2_sb[kt][:, mt * 128 : (mt + 1) * 128],
                            rhs=h1[kt][:],
                            start=(kt == 0), stop=(kt == 3),
                        )
                    h = hp.tile([128, NB], BF16, tag=f"h2{mt}")
                    nc.scalar.activation(
                        h[:], ps[:], mybir.ActivationFunctionType.Tanh,
                        bias=b2T[:, mt : mt + 1],
                    )
                    h2.append(h)

                # MLP layer 3: 256 -> 128, tanh
                ps = psmm.tile([128, NB], F32, tag="mmA")
                for kt in range(2):
                    nc.tensor.matmul(
                        ps[:],
                        lhsT=w3_sb[kt][:],
                        rhs=h2[kt][:],
                        start=(kt == 0), stop=(kt == 1),
                    )
                h3 = hp.tile([128, NB], BF16, tag="h3")
                nc.scalar.activation(
                    h3[:], ps[:], mybir.ActivationFunctionType.Tanh, bias=b3T[:, 0:1]
                )

                # regression head: 128 -> 1, + br + 3.5
                pp = psmm.tile([128, NB], F32, tag="mmB")
                nc.tensor.matmul(
                    pp[:1, :], lhsT=wr_sb[:, :1], rhs=h3[:],
                    start=True, stop=True,
                )
                pred = op.tile([1, NB], F32, tag="pred")
                nc.vector.tensor_scalar(
                    out=pred[:], in0=pp[:1, :], scalar1=br_sb[:1, 0:1],
                    scalar2=GLOBAL_AVG, op0=mm.add, op1=mm.add,
                )
                nc.sync.dma_start(out=out_d[t * NB : (t + 1) * NB], in_=pred[:1, :])

    if fix_drains:
        _fix_drains(nc)
    return nc


def _bucketize(rows, cols, n_cores=N_CORES):
    """Sort the batch by (user_chunk, item_chunk), pad each bucket to a
    multiple of n_cores*128 (and the total group count to a multiple of
    4 per core), then deal equal 128-row groups to each core.

    Returns groups [(cu, ci)] per group (shared by all cores), per-core
    relative int16 indices u16/i16 [n_cores, bc], and per-core original
    positions pos [n_cores, bc] (-1 for padding)."""
    rows = np.asarray(rows, np.int64)
    cols = np.asarray(cols, np.int64)
    cu = rows // CHUNK
    ci = cols // CHUNK
    b = cu * 2 + ci
    order = np.argsort(b, kind="stable")
    BLK = n_cores * 128

    seq_pos, seq_u, seq_i, blk_bucket = [], [], [], []

    def emit(idx, bk, npad):
        seq_pos.append(idx)
        seq_u.append(rows[idx] - (bk // 2) * CHUNK)
        seq_i.append(cols[idx] - (bk % 2) * CHUNK)
        if npad:
            seq_pos.append(np.full(npad, -1, np.int64))
            seq_u.append(np.zeros(npad, np.int64))
            seq_i.append(np.zeros(npad, np.int64))
        blk_bucket.extend([bk] * ((len(idx) + npad) // BLK))

    for bk in range(8):
        idx = order[b[order] == bk]
        if len(idx) == 0:
            continue
        emit(idx, bk, (-len(idx)) % BLK)
    # total groups per core must be a multiple of 4 (NB=512 batch tiles)
    extra = (-len(blk_bucket)) % 4
    for _ in range(extra):
        emit(np.empty(0, np.int64), 0, BLK)

    pos = np.concatenate(seq_pos)
    u_rel = np.concatenate(seq_u).astype(np.int16)
    i_rel = np.concatenate(seq_i).astype(np.int16)
    n_blocks = len(pos) // BLK
    groups = [(bk // 2, bk % 2) for bk in blk_bucket]

    def deal(arr):
        return np.ascontiguousarray(
            arr.reshape(n_blocks, n_cores, 128).transpose(1, 0, 2).reshape(n_cores, -1)
        )

    return groups, deal(u_rel), deal(i_rel), deal(pos)


def _wrap16(v):
    """[bc] int16 -> [128, bc//16] gather-index layout (idx j at partition
    j%16, col j//16; replicated across the 8 16-partition lanes)."""
    t = v.reshape(-1, 16).T  # [16, bc//16]
    return np.ascontiguousarray(np.tile(t, (8, 1)))


def _host_prep(rows, cols, user_inter, item_inter, user_indep_x, item_indep_x,
               Wt, bt, W1, b1, W2, b2, W3, b3, Wr, br, n_cores=N_CORES):
    """Returns (groups, in_maps, pos) — pos for un-permuting the output."""
    import ml_dtypes
    md = ml_dtypes.bfloat16
    f32 = np.float32

    def pack(inter, indep):
        n = inter.shape[0]
        tab = np.zeros((n, DIM_P), md)
        tab[:, :DIM_C] = np.asarray(inter, f32)
        tab[:, DIM_C : DIM_C + DIM_S] = np.asarray(indep, f32)
        return tab

    tab_u = pack(user_inter, user_indep_x)
    tab_i = pack(item_inter, item_indep_x)
    wtT = np.ascontiguousarray(np.asarray(Wt, f32).T.astype(md))
    # factor layout is [inter(960), u_s(32), i_s(32)] -> permute W1 columns
    W1 = np.asarray(W1, f32)
    w1p = np.concatenate([W1[:, 64:], W1[:, :32], W1[:, 32:64]], axis=1)
    w1T = np.ascontiguousarray(w1p.T.astype(md))
    w2T = np.ascontiguousarray(np.asarray(W2, f32).T.astype(md))
    w3T = np.ascontiguousarray(np.asarray(W3, f32).T.astype(md))
    wrT = np.ascontiguousarray(np.asarray(Wr, f32).T.astype(md))

    def padT(v, ntiles):
        v = np.asarray(v, f32)
        out = np.zeros((128, ntiles), f32)
        for ti in range(ntiles):
            seg = v[ti * 128 : (ti + 1) * 128]
            out[: len(seg), ti] = seg
        return out

    shared = dict(tab_u=tab_u, tab_i=tab_i, wtT=wtT, w1T=w1T, w2T=w2T, w3T=w3T,
                  wrT=wrT, btT=padT(bt, 8), b1T=padT(b1, 4), b2T=padT(b2, 2),
                  b3T=padT(b3, 1), br=np.asarray(br, f32).reshape(1, 1))

    groups, u16, i16, pos = _bucketize(rows, cols, n_cores)
    in_maps = []
    for c in range(n_cores):
        m = dict(shared)
        m["rows16"] = _wrap16(u16[c])
        m["cols16"] = _wrap16(i16[c])
        in_maps.append(m)
    return groups, in_maps, pos


def kernel(rows, cols, user_inter, item_inter, user_indep_x, item_indep_x,
           Wt, bt, W1, b1, W2, b2, W3, b3, Wr, br):
    groups, in_maps, pos = _host_prep(
        rows, cols, user_inter, item_inter, user_indep_x, item_indep_x,
        Wt, bt, W1, b1, W2, b2, W3, b3, Wr, br)
    nc = build_nc(groups)
    res = run_bass_kernel_spmd(nc, in_maps, list(range(N_CORES)))
    flat = np.stack([res.results[c]["out"] for c in range(N_CORES)])  # [8, bc]
    out = np.empty(BATCH, np.float32)
    p = pos.reshape(-1)
    v = flat.reshape(-1)
    valid = p >= 0
    out[p[valid]] = v[valid]
    return out.reshape(BATCH, 1)
